# revision 1
# baseline (speedup 1.0000x reference)
"""Trainium2 Bass kernel for causal GQA self-attention (B=2,S=2048,D=1024,H=16,HKV=4,HD=64).

Sharding: 8 cores = DP(2 over batch) x TP(4 over GQA groups).
Each core computes, for one batch element and one GQA group (4 q heads + 1 kv head),
the partial output  y_group @ Wo[:, group_cols].T  (row-sharded Wo).
Host sums the 4 TP partials per batch element.
"""

import sys
from contextlib import ExitStack

sys.path.insert(0, "/opt/trn_rl_repo")

import numpy as np
import ml_dtypes

import concourse.bass as bass
import concourse.bacc as bacc
import concourse.tile as tile
import concourse.mybir as mybir
from concourse.bass_utils import run_bass_kernel_spmd

BF16 = mybir.dt.bfloat16
F32 = mybir.dt.float32
AF = mybir.ActivationFunctionType
BF16NP = ml_dtypes.bfloat16

D, H, HKV, HD, B, S = 1024, 16, 4, 64, 2, 2048
HG = 4              # q heads per core
KV_DIM = HKV * HD   # 256
E = HG * HD         # 256 local q-proj dim
ROPE_BASE = 10000.0
EPS = float(np.finfo(np.float32).eps)

import os
KPHASE = int(os.environ.get("KPHASE", "3"))
KDEBUG = int(os.environ.get("KDEBUG", "0"))

NK = D // 128       # 8 contraction tiles for qkv projections
SQB = 256           # sq block size in attention
NB = S // SQB       # 8 blocks
NJ = S // 128       # 16 sk tiles
NS5 = S // 512      # 4 n-tiles of 512 in projections


def _consts():
    """Constant tensors baked into the NEFF (same for every core)."""
    i = np.arange(32, dtype=np.float64)
    inv_freq = 1.0 / (ROPE_BASE ** (2.0 * i / HD))
    pos = np.arange(S, dtype=np.float64)
    fr = pos[:, None] * inv_freq[None, :]          # [S, 32]
    cosT = np.cos(fr).T.astype(np.float32)          # [32, S]
    sinT = np.sin(fr).T.astype(np.float32)
    cos4 = np.tile(cosT, (4, 1)).astype(BF16NP)     # [128, S]
    sin4 = np.tile(sinT, (4, 1)).astype(BF16NP)
    nsin4 = (-np.tile(sinT, (4, 1))).astype(BF16NP)

    # causal masks for diagonal sk-tiles: pattern p in {0,1}
    # valid iff c >= 128*p + r   (r: sk row 0..127, c: sq col 0..255)
    r = np.arange(128)[:, None]
    c = np.arange(SQB)[None, :]
    masks = []
    for p in range(2):
        m = (c >= 128 * p + r).astype(BF16NP)       # [128, 256]
        masks.append(np.tile(m, (1, HG)))            # [128, 1024] (4 head blocks)

    bsel4 = np.zeros((4, 128), dtype=BF16NP)        # broadcast f[h] -> rows 32h..32h+32
    for h in range(4):
        bsel4[h, 32 * h:32 * h + 32] = 1.0
    sel4 = bsel4.T.copy()                            # [128, 4] sumsq selector
    ones64 = np.ones((1, 64), dtype=BF16NP)
    ones64col = np.ones((64, 1), dtype=BF16NP)
    id128 = np.eye(128, dtype=BF16NP)
    return cos4, sin4, nsin4, masks, bsel4, sel4, ones64, ones64col, id128


def _build():
    nc = bacc.Bacc("TRN2", debug=False)

    xT_d = nc.dram_tensor("xT", [D, S], BF16, kind="ExternalInput")
    wq_d = nc.dram_tensor("wq", [NK, 128, E], BF16, kind="ExternalInput")
    wkv_d = nc.dram_tensor("wkv", [NK, 128, 128], BF16, kind="ExternalInput")
    wo_d = nc.dram_tensor("wo", [2, 128, D], BF16, kind="ExternalInput")
    qlnb_d = nc.dram_tensor("qlnb", [4, 1], F32, kind="ExternalInput")
    out_d = nc.dram_tensor("out", [S, D], F32, kind="ExternalOutput")
    dbg = {}
    if KDEBUG:
        for nm, shp in [("d_qsb0", [128, S]), ("d_qsb1", [128, S]),
                        ("d_kvsb", [128, S]), ("d_fq", [4, S]), ("d_fbcq", [128, S]),
                        ("d_qstd0", [128, S]), ("d_qstd1", [128, S]),
                        ("d_kdup", [128, S]), ("d_vsb", [128, NJ, 65]),
                        ("d_yn0", [128, S]), ("d_yn1", [128, S]),
                        ("d_pt0", [128, HG * SQB]), ("d_pt1", [128, HG * SQB]),
                        ("d_yt", [128, 4 * 256]), ("d_rbs", [128, 4 * 256])]:
            dbg[nm] = nc.dram_tensor(nm, shp, BF16, kind="ExternalOutput")

    cos4, sin4, nsin4, masks, bsel4, sel4, ones64, ones64col, id128 = _consts()
    cos4_d = nc.inline_tensor(cos4, "cos4")
    sin4_d = nc.inline_tensor(sin4, "sin4")
    nsin4_d = nc.inline_tensor(nsin4, "nsin4")
    mask_d = [nc.inline_tensor(masks[p], f"mask{p}") for p in range(2)]
    bsel4_d = nc.inline_tensor(bsel4, "bsel4")
    sel4_d = nc.inline_tensor(sel4, "sel4")
    ones64_d = nc.inline_tensor(ones64, "ones64")
    ones64col_d = nc.inline_tensor(ones64col, "ones64col")
    id128_d = nc.inline_tensor(id128, "id128")

    with tile.TileContext(nc) as tc, ExitStack() as ctx:
        sp = ctx.enter_context(tc.tile_pool(name="static", bufs=1))

        def stile(shape, dt, tag):
            return sp.tile(shape, dt, name=tag, tag=tag)

        # ---- static SBUF tensors ----
        xt = [stile([128, S], BF16, f"xt{k}") for k in range(NK)]
        wq = stile([128, NK, E], BF16, "wq")
        wkv = stile([128, NK, 128], BF16, "wkv")
        wo = stile([128, 2, D], BF16, "wo")
        cos4_s = stile([128, S], BF16, "cos4")
        sin4_s = stile([128, S], BF16, "sin4")
        nsin4_s = stile([128, S], BF16, "nsin4")
        mask_s = [stile([128, HG * SQB], BF16, f"mask{p}") for p in range(2)]
        bsel4_s = stile([4, 128], BF16, "bsel4")
        sel4_s = stile([128, 4], BF16, "sel4")
        ones64_s = stile([1, 64], BF16, "ones64")
        ones64col_s = stile([64, 1], BF16, "ones64col")
        id128_s = stile([128, 128], BF16, "id128")
        qlnb_s = stile([4, 1], F32, "qlnb")
        epsb = stile([128, 1], F32, "epsb")
        zb = stile([128, 1], F32, "zb")

        qsb = [stile([128, S], BF16, f"qsb{m}") for m in range(2)]   # T/B packed
        kvsb = stile([128, S], BF16, "kvsb")                          # k(0:64) | v(64:128)
        sqq = [stile([128, S], BF16, f"sqq{m}") for m in range(2)]
        sqkv = stile([64, S], BF16, "sqkv")
        fq = stile([4, S], BF16, "fq")
        fk = stile([1, S], BF16, "fk")
        fbcq = stile([128, S], BF16, "fbcq")
        fbck = stile([64, S], BF16, "fbck")
        qr = [stile([128, S], BF16, f"qr{m}") for m in range(2)]      # rotated T/B
        kr = [stile([32, S], BF16, f"kr{m}") for m in range(2)]
        qstd = [stile([128, S], BF16, f"qstd{m}") for m in range(2)]  # per-head layout
        kdup = stile([128, S], BF16, "kdup")
        kb0 = stile([32, S], BF16, "kb0")
        onesq = stile([128, 64], BF16, "onesq")
        vsb = stile([128, NJ, 65], BF16, "vsb")                       # [v | ones]
        yn = [stile([128, S], BF16, f"yn{m}") for m in range(2)]      # normalized y^T

        # ---- load everything ----
        for k in range(NK):
            nc.sync.dma_start(xt[k][:], xT_d[128 * k:128 * (k + 1), :])
            nc.sync.dma_start(wq[:, k, :], wq_d[k])
            nc.sync.dma_start(wkv[:, k, :], wkv_d[k])
        nc.sync.dma_start(wo[:, 0, :], wo_d[0])
        nc.sync.dma_start(wo[:, 1, :], wo_d[1])
        nc.sync.dma_start(cos4_s[:], cos4_d[:])
        nc.sync.dma_start(sin4_s[:], sin4_d[:])
        nc.sync.dma_start(nsin4_s[:], nsin4_d[:])
        for p in range(2):
            nc.sync.dma_start(mask_s[p][:], mask_d[p][:])
        nc.sync.dma_start(bsel4_s[:], bsel4_d[:])
        nc.sync.dma_start(sel4_s[:], sel4_d[:])
        nc.sync.dma_start(ones64_s[:], ones64_d[:])
        nc.sync.dma_start(ones64col_s[:], ones64col_d[:])
        nc.sync.dma_start(id128_s[:], id128_d[:])
        nc.sync.dma_start(qlnb_s[:], qlnb_d[:])
        nc.vector.memset(vsb[:], 1.0)  # ones column at [:, j, 64]; 0:64 overwritten below
        nc.vector.memset(epsb[:], EPS)
        nc.vector.memset(zb[:], 0.0)
        nc.vector.memset(onesq[:], 1.0)

        # ======== phase 1: projections + rms factors + rope ========
        with (
            tc.tile_pool(name="pp", bufs=4, space=bass.MemorySpace.PSUM) as pp,
            tc.tile_pool(name="lns", bufs=2) as lns,
        ):
            # Q projection -> qsb (permuted: tileT = tops of 4 heads, tileB = bottoms)
            for m in range(2):
                pq = [pp.tile([128, 512], F32, name="pq", tag="pq", bufs=4) for _ in range(NS5)]
                for k in range(NK):
                    for n in range(NS5):
                        nc.tensor.matmul(
                            pq[n][:], wq[:, k, 128 * m:128 * (m + 1)],
                            xt[k][:, 512 * n:512 * (n + 1)],
                            start=(k == 0), stop=(k == NK - 1))
                for n in range(NS5):
                    sl = slice(512 * n, 512 * (n + 1))
                    nc.scalar.copy(qsb[m][:, sl], pq[n][:])
                    nc.vector.tensor_mul(sqq[m][:, sl], qsb[m][:, sl], qsb[m][:, sl])
            # KV projection
            pkv = [pp.tile([128, 512], F32, name="pq", tag="pq", bufs=4) for _ in range(NS5)]
            for k in range(NK):
                for n in range(NS5):
                    nc.tensor.matmul(
                        pkv[n][:], wkv[:, k, :], xt[k][:, 512 * n:512 * (n + 1)],
                        start=(k == 0), stop=(k == NK - 1))
            for n in range(NS5):
                sl = slice(512 * n, 512 * (n + 1))
                nc.scalar.copy(kvsb[:, sl], pkv[n][:])
                nc.vector.tensor_mul(sqkv[:, sl], kvsb[0:64, sl], kvsb[0:64, sl])
                # v transpose: [64,128] slices -> [128,64]
                for t in range(4):
                    st = 4 * n + t
                    ptr = pp.tile([128, 64], BF16, name="ptr", tag="ptr", bufs=2)
                    nc.tensor.transpose(
                        ptr[:], kvsb[64:128, 128 * st:128 * (st + 1)],
                        id128_s[64:128, 64:128])
                    nc.vector.tensor_copy(vsb[:, st, 0:64], ptr[:])

            # rms factors: f = exp(-0.5*ln(ssq/HD + eps) + ln(gain/8))
            for n in range(NS5):
                sl = slice(512 * n, 512 * (n + 1))
                psq = pp.tile([4, 512], F32, name="psq", tag="psq", bufs=2)
                nc.tensor.matmul(psq[:], sel4_s[:], sqq[0][:, sl], start=True, stop=False)
                nc.tensor.matmul(psq[:], sel4_s[:], sqq[1][:, sl], start=False, stop=True)
                lnt = lns.tile([4, 512], F32, name="pln", tag="pln")
                nc.scalar.activation(lnt[:], psq[:], AF.Ln, scale=1.0 / HD, bias=epsb[0:4, :])
                nc.scalar.activation(fq[:, sl], lnt[:], AF.Exp, scale=-0.5,
                                     bias=qlnb_s[:, :])
                psk = pp.tile([1, 512], F32, name="psq", tag="psq", bufs=2)
                nc.tensor.matmul(psk[:], ones64col_s[:], sqkv[:, sl], start=True, stop=True)
                lnk = lns.tile([1, 512], F32, name="pln", tag="pln")
                nc.scalar.activation(lnk[:], psk[:], AF.Ln, scale=1.0 / HD, bias=epsb[0:1, :])
                nc.scalar.activation(fk[:, sl], lnk[:], AF.Exp, scale=-0.5, bias=zb[0:1, :])
                # broadcast factors along hd rows via PE
                pb = pp.tile([128, 512], F32, name="pq", tag="pq", bufs=4)
                nc.tensor.matmul(pb[:], bsel4_s[:], fq[:, sl], start=True, stop=True)
                nc.scalar.copy(fbcq[:, sl], pb[:])
                pbk = pp.tile([64, 512], F32, name="pq", tag="pq", bufs=4)
                nc.tensor.matmul(pbk[:], ones64_s[:], fk[:, sl], start=True, stop=True)
                nc.scalar.copy(fbck[:, sl], pbk[:])

            # k bottom half shifted to partition base 0 (DVE ops need aligned bases)
            nc.sync.dma_start(kb0[:], kvsb[32:64, :])

            # rope + scale (DVE, bf16)
            with tc.tile_pool(name="rt", bufs=4) as rt:
                for n in range(NS5):
                    sl = slice(512 * n, 512 * (n + 1))
                    t1 = rt.tile([128, 512], BF16, name="t1", tag="t1")
                    t2 = rt.tile([128, 512], BF16, name="t2", tag="t2")
                    nc.vector.tensor_mul(t1[:], qsb[0][:, sl], cos4_s[:, sl])
                    nc.vector.tensor_mul(t2[:], qsb[1][:, sl], sin4_s[:, sl])
                    nc.vector.tensor_add(t1[:], t1[:], t2[:])
                    nc.vector.tensor_mul(qr[0][:, sl], t1[:], fbcq[:, sl])
                    u1 = rt.tile([128, 512], BF16, name="t1", tag="t1")
                    u2 = rt.tile([128, 512], BF16, name="t2", tag="t2")
                    nc.vector.tensor_mul(u1[:], qsb[0][:, sl], nsin4_s[:, sl])
                    nc.vector.tensor_mul(u2[:], qsb[1][:, sl], cos4_s[:, sl])
                    nc.vector.tensor_add(u1[:], u1[:], u2[:])
                    nc.vector.tensor_mul(qr[1][:, sl], u1[:], fbcq[:, sl])
                    k1 = rt.tile([32, 512], BF16, name="k1", tag="k1")
                    k2 = rt.tile([32, 512], BF16, name="k2", tag="k2")
                    nc.vector.tensor_mul(k1[:], kvsb[0:32, sl], cos4_s[0:32, sl])
                    nc.vector.tensor_mul(k2[:], kb0[:, sl], sin4_s[0:32, sl])
                    nc.vector.tensor_add(k1[:], k1[:], k2[:])
                    nc.vector.tensor_mul(kr[0][:, sl], k1[:], fbck[0:32, sl])
                    k3 = rt.tile([32, 512], BF16, name="k1", tag="k1")
                    k4 = rt.tile([32, 512], BF16, name="k2", tag="k2")
                    nc.vector.tensor_mul(k3[:], kvsb[0:32, sl], nsin4_s[0:32, sl])
                    nc.vector.tensor_mul(k4[:], kb0[:, sl], cos4_s[0:32, sl])
                    nc.vector.tensor_add(k3[:], k3[:], k4[:])
                    nc.vector.tensor_mul(kr[1][:, sl], k3[:], fbck[0:32, sl])

        # reassemble per-head layout (DMA partition moves)
        for h in range(4):
            dst = qstd[h // 2]
            base = 64 * (h % 2)
            nc.sync.dma_start(dst[base:base + 32, :], qr[0][32 * h:32 * h + 32, :])
            nc.sync.dma_start(dst[base + 32:base + 64, :], qr[1][32 * h:32 * h + 32, :])
        nc.sync.dma_start(kdup[0:32, :], kr[0][:])
        nc.sync.dma_start(kdup[32:64, :], kr[1][:])
        nc.sync.dma_start(kdup[64:96, :], kr[0][:])
        nc.sync.dma_start(kdup[96:128, :], kr[1][:])

        # ======== phase 2: attention ========
        if KPHASE >= 2:
            with (
              tc.tile_pool(name="ps", bufs=2, space=bass.MemorySpace.PSUM) as ps,
              tc.tile_pool(name="py", bufs=4, space=bass.MemorySpace.PSUM) as py,
              tc.tile_pool(name="pa", bufs=3) as pa,
          ):
              for b in range(NB):
                  sq = slice(SQB * b, SQB * (b + 1))
                  jmax = 2 * b + 1
                  yt = [py.tile([65, 256], F32, name="yt", tag="yt") for _ in range(4)]
                  # concurrent row-group pairs (h even @rows 0:64, h odd @64:128)
                  # must hit different PSUM banks: head h -> col COLOF[h].
                  COLOF = [0, 512, 256, 768]
                  for j in range(jmax + 1):
                      stile_ = ps.tile([128, HG * SQB], F32, name="st", tag="st")
                      for h in range(4):
                          base = 64 * (h % 2)
                          co = COLOF[h]
                          nc.tensor.matmul(
                              stile_[:, co:co + SQB],
                              kdup[base:base + 64, 128 * j:128 * (j + 1)],
                              qstd[h // 2][base:base + 64, sq],
                              start=True, stop=True, skip_group_check=True)
                      pt = pa.tile([128, HG * SQB], BF16, name="pt", tag="pt")
                      nc.scalar.activation(pt[:], stile_[:], AF.Exp, bias=zb[:, :])
                      if j >= 2 * b:
                          nc.vector.tensor_mul(pt[:], pt[:], mask_s[j - 2 * b][:])
                      if KDEBUG and b == 0:
                          nc.sync.dma_start(dbg[f"d_pt{j}"][:], pt[:])
                      for h in range(4):
                          nc.tensor.matmul(
                              yt[h][:], vsb[:, j, :], pt[:, COLOF[h]:COLOF[h] + SQB],
                              start=(j == 0), stop=(j == jmax))
                  if KDEBUG and b == 0:
                      for h in range(4):
                          ytc = pa.tile([128, 256], BF16, name="ytc", tag="ytc")
                          nc.vector.tensor_copy(ytc[0:65, :], yt[h][:])
                          nc.sync.dma_start(dbg["d_yt"][:, 256 * h:256 * (h + 1)],
                                            ytc[:])
                  # normalize: y / denom, write into yn (per-head rows)
                  for h in range(4):
                      dcb = pa.tile([128, 256], BF16, name="dcb", tag="dcb")
                      nc.vector.tensor_copy(dcb[64:65, :], yt[h][64:65, :])
                      prb = ps.tile([64, 256], F32, name="st", tag="st")
                      nc.tensor.matmul(prb[:], onesq[64:65, :], dcb[64:65, :],
                                       start=True, stop=True)
                      dbs = pa.tile([64, 256], F32, name="dbs", tag="dbs")
                      nc.vector.tensor_copy(dbs[:], prb[:])
                      rbs = pa.tile([64, 256], F32, name="rbs", tag="rbs")
                      nc.vector.reciprocal_approx_fast(rbs[:], dbs[:])
                      if KDEBUG and b == 0:
                          rbc_ = pa.tile([128, 256], BF16, name="rbc_", tag="ytc")
                          nc.vector.tensor_copy(rbc_[0:64, :], rbs[:])
                          nc.sync.dma_start(dbg["d_rbs"][:, 256 * h:256 * (h + 1)],
                                            rbc_[:])
                      if h % 2 == 0:
                          nc.vector.tensor_mul(yn[h // 2][0:64, sq],
                                               yt[h][0:64, :], rbs[:])
                      else:
                          stg = pa.tile([64, 256], BF16, name="stg", tag="stg")
                          nc.vector.tensor_mul(stg[:], yt[h][0:64, :], rbs[:])
                          nc.sync.dma_start(yn[h // 2][64:128, sq], stg[:])

        if KDEBUG:
                nc.sync.dma_start(dbg["d_qsb0"][:], qsb[0][:])
                nc.sync.dma_start(dbg["d_qsb1"][:], qsb[1][:])
                nc.sync.dma_start(dbg["d_kvsb"][:], kvsb[:])
                nc.sync.dma_start(dbg["d_fq"][:], fq[:])
                nc.sync.dma_start(dbg["d_fbcq"][:], fbcq[:])
                nc.sync.dma_start(dbg["d_qstd0"][:], qstd[0][:])
                nc.sync.dma_start(dbg["d_qstd1"][:], qstd[1][:])
                nc.sync.dma_start(dbg["d_kdup"][:], kdup[:])
                nc.sync.dma_start(dbg["d_vsb"][:], vsb[:])
                if KPHASE >= 2:
                    nc.sync.dma_start(dbg["d_yn0"][:], yn[0][:])
                    nc.sync.dma_start(dbg["d_yn1"][:], yn[1][:])

        # ======== phase 3: output projection ========
        if KPHASE >= 3:
            with (
              tc.tile_pool(name="po", bufs=2, space=bass.MemorySpace.PSUM) as po,
              tc.tile_pool(name="ob", bufs=3) as ob,
          ):
              for st in range(16):
                  ssl = slice(128 * st, 128 * (st + 1))
                  pot = po.tile([128, D], F32, name="po", tag="po")
                  for n in range(2):
                      for kk in range(2):
                          nc.tensor.matmul(
                              pot[:, 512 * n:512 * (n + 1)], yn[kk][:, ssl],
                              wo[:, kk, 512 * n:512 * (n + 1)],
                              start=(kk == 0), stop=(kk == 1))
                  ot = ob.tile([128, D], F32, name="ot", tag="ot")
                  nc.vector.tensor_copy(ot[:], pot[:])
                  nc.sync.dma_start(out_d[ssl, :], ot[:])

    nc.finalize()
    return nc


_NC = None


def _get_nc():
    global _NC
    if _NC is None:
        _NC = _build()
    return _NC


def _perm():
    tops = [h * 64 + i for h in range(HG) for i in range(32)]
    bots = [h * 64 + 32 + i for h in range(HG) for i in range(32)]
    return tops + bots


def kernel(x, Wq, Wk, Wv, Wo, q_gain):
    x = np.asarray(x, dtype=np.float32)
    Wq = np.asarray(Wq, dtype=np.float32)
    Wk = np.asarray(Wk, dtype=np.float32)
    Wv = np.asarray(Wv, dtype=np.float32)
    Wo = np.asarray(Wo, dtype=np.float32)
    q_gain = np.asarray(q_gain, dtype=np.float32)

    perm = _perm()
    in_maps = []
    for c in range(8):
        dp, tp = divmod(c, 4)
        xT = np.ascontiguousarray(x[dp].T).astype(BF16NP)
        wq_sel = Wq[tp * E:(tp + 1) * E].T[:, perm]          # [D, 256] permuted
        wq_t = np.ascontiguousarray(wq_sel).astype(BF16NP).reshape(NK, 128, E)
        wk_sel = Wk[tp * HD:(tp + 1) * HD].T                  # [D, 64]
        wv_sel = Wv[tp * HD:(tp + 1) * HD].T
        wkv_t = np.concatenate([wk_sel, wv_sel], axis=1).astype(BF16NP)
        wkv_t = np.ascontiguousarray(wkv_t).reshape(NK, 128, 128)
        wo_sel = Wo[:, tp * E:(tp + 1) * E].T                 # [256, D]
        wo_t = np.ascontiguousarray(wo_sel).astype(BF16NP).reshape(2, 128, D)
        g = q_gain[tp * HG:(tp + 1) * HG].astype(np.float64)
        qlnb = np.log(np.maximum(g, 1e-30) / 8.0).astype(np.float32).reshape(4, 1)
        in_maps.append({
            "xT": xT, "wq": wq_t, "wkv": wkv_t, "wo": wo_t, "qlnb": qlnb,
        })

    nc = _get_nc()
    res = run_bass_kernel_spmd(nc, in_maps, core_ids=list(range(8)))
    out = np.zeros((B, S, D), dtype=np.float32)
    for c in range(8):
        out[c // 4] += res.results[c]["out"]
    return out



# revision 6
# speedup vs baseline: 223.4603x; 223.4603x over previous
"""Trainium2 Bass kernel for causal GQA self-attention (B=2,S=2048,D=1024,H=16,HKV=4,HD=64).

Sharding: 8 cores = DP(2 over batch) x TP(4 over GQA groups).
Each core computes, for one batch element and one GQA group (4 q heads + 1 kv head),
the partial output  y_group @ Wo[:, group_cols].T  (row-sharded Wo).
Host sums the 4 TP partials per batch element.

v2: batched input DMAs, N=512 attention matmuls (head pairs packed in columns),
interleaved per-block output projection, ScalarE reserved for softmax exp,
GpSimd denominator broadcast, deeper PSUM pipelining.
"""

import sys
from contextlib import ExitStack

sys.path.insert(0, "/opt/trn_rl_repo")

import numpy as np
import ml_dtypes

import concourse.bass as bass
import concourse.bacc as bacc
import concourse.tile as tile
import concourse.mybir as mybir
from concourse.bass_utils import run_bass_kernel_spmd

BF16 = mybir.dt.bfloat16
F32 = mybir.dt.float32
AF = mybir.ActivationFunctionType
BF16NP = ml_dtypes.bfloat16

D, H, HKV, HD, B, S = 1024, 16, 4, 64, 2, 2048
HG = 4              # q heads per core
KV_DIM = HKV * HD   # 256
E = HG * HD         # 256 local q-proj dim
ROPE_BASE = 10000.0
EPS = float(np.finfo(np.float32).eps)

NK = D // 128       # 8 contraction tiles for qkv projections
SQB = 256           # sq block size in attention
NB = S // SQB       # 8 blocks
NJ = S // 128       # 16 sk tiles
NS5 = S // 512      # 4 n-tiles of 512 in projections

# const block column offsets (bf16 [128, CW])
_CO_COS = 0
_CO_SIN = _CO_COS + S
_CO_NSIN = _CO_SIN + S
_CO_M0 = _CO_NSIN + S
_CO_M1 = _CO_M0 + HG * SQB
_CO_ID = _CO_M1 + HG * SQB
_CO_SEL = _CO_ID + 128          # sel4 [128,4]
_CO_BSEL = _CO_SEL + 4          # bsel4 [4,128]
_CO_O64C = _CO_BSEL + 128       # ones64col [64,1]
_CO_O64R = _CO_O64C + 1         # ones64 row [1,64]
CW = _CO_O64R + 64


def _consts():
    """Constant block baked into the NEFF (same for every core): [128, CW] bf16."""
    blk = np.zeros((128, CW), dtype=BF16NP)
    i = np.arange(32, dtype=np.float64)
    inv_freq = 1.0 / (ROPE_BASE ** (2.0 * i / HD))
    pos = np.arange(S, dtype=np.float64)
    fr = pos[:, None] * inv_freq[None, :]          # [S, 32]
    cosT = np.cos(fr).T.astype(np.float32)          # [32, S]
    sinT = np.sin(fr).T.astype(np.float32)
    blk[:, _CO_COS:_CO_COS + S] = np.tile(cosT, (4, 1)).astype(BF16NP)
    blk[:, _CO_SIN:_CO_SIN + S] = np.tile(sinT, (4, 1)).astype(BF16NP)
    blk[:, _CO_NSIN:_CO_NSIN + S] = (-np.tile(sinT, (4, 1))).astype(BF16NP)

    # causal masks for diagonal sk-tiles: pattern p in {0,1}
    # valid iff c >= 128*p + r   (r: sk row 0..127, c: sq col 0..255)
    r = np.arange(128)[:, None]
    c = np.arange(SQB)[None, :]
    for p, co in ((0, _CO_M0), (1, _CO_M1)):
        m = (c >= 128 * p + r).astype(BF16NP)       # [128, 256]
        blk[:, co:co + HG * SQB] = np.tile(m, (1, HG))

    blk[:, _CO_ID:_CO_ID + 128] = np.eye(128, dtype=BF16NP)
    sel4 = np.zeros((128, 4), dtype=BF16NP)         # sumsq selector: tops of head h
    for h in range(4):
        sel4[32 * h:32 * h + 32, h] = 1.0
    blk[:, _CO_SEL:_CO_SEL + 4] = sel4
    bsel4 = np.zeros((4, 128), dtype=BF16NP)        # broadcast f[h] -> rows 32h..32h+32
    for h in range(4):
        bsel4[h, 32 * h:32 * h + 32] = 1.0
    blk[0:4, _CO_BSEL:_CO_BSEL + 128] = bsel4
    blk[0:64, _CO_O64C] = 1.0                       # ones64col [64,1]
    blk[0:1, _CO_O64R:_CO_O64R + 64] = 1.0          # ones64 row [1,64]
    return blk


def _build():
    nc = bacc.Bacc("TRN2", debug=False)

    xt_d = nc.dram_tensor("xt", [128, NK * S], BF16, kind="ExternalInput")
    wq_d = nc.dram_tensor("wq", [128, NK * E], BF16, kind="ExternalInput")
    wkv_d = nc.dram_tensor("wkv", [128, NK * 128], BF16, kind="ExternalInput")
    wo_d = nc.dram_tensor("wo", [128, 2 * D], BF16, kind="ExternalInput")
    qlnb_d = nc.dram_tensor("qlnb", [4, 1], F32, kind="ExternalInput")
    out_d = nc.dram_tensor("out", [S, D], F32, kind="ExternalOutput")

    cblk_d = nc.inline_tensor(_consts(), "cblk")

    with tile.TileContext(nc) as tc, ExitStack() as ctx:
        sp = ctx.enter_context(tc.tile_pool(name="static", bufs=1))

        def stile(shape, dt, tag):
            return sp.tile(shape, dt, name=tag, tag=tag)

        # ---- static SBUF tensors ----
        xt = stile([128, NK * S], BF16, "xt")
        wq = stile([128, NK * E], BF16, "wq")
        wkv = stile([128, NK * 128], BF16, "wkv")
        wo = stile([128, 2 * D], BF16, "wo")
        cb = stile([128, CW], BF16, "cb")
        qlnb_s = stile([4, 1], F32, "qlnb")
        epsb = stile([128, 1], F32, "epsb")
        zb = stile([128, 1], F32, "zb")

        # const views
        cos4 = cb[:, _CO_COS:_CO_COS + S]
        sin4 = cb[:, _CO_SIN:_CO_SIN + S]
        nsin4 = cb[:, _CO_NSIN:_CO_NSIN + S]
        mask_s = [cb[:, _CO_M0:_CO_M0 + HG * SQB], cb[:, _CO_M1:_CO_M1 + HG * SQB]]
        id128 = cb[:, _CO_ID:_CO_ID + 128]
        sel4 = cb[:, _CO_SEL:_CO_SEL + 4]
        bsel4 = cb[0:4, _CO_BSEL:_CO_BSEL + 128]
        ones64col = cb[0:64, _CO_O64C:_CO_O64C + 1]
        ones64row = cb[0:1, _CO_O64R:_CO_O64R + 64]

        qsb = [stile([128, S], BF16, f"qsb{m}") for m in range(2)]   # T/B packed
        kvsb = stile([128, S], BF16, "kvsb")                          # k(0:64) | v(64:128)
        sqq = [stile([128, S], BF16, f"sqq{m}") for m in range(2)]
        sqkv = stile([64, S], BF16, "sqkv")
        fq = stile([4, S], BF16, "fq")
        fk = stile([1, S], BF16, "fk")
        fbcq = stile([128, S], BF16, "fbcq")
        fbck = stile([64, S], BF16, "fbck")
        qr = [stile([128, S], BF16, f"qr{m}") for m in range(2)]      # rotated T/B
        kr = [stile([32, S], BF16, f"kr{m}") for m in range(2)]
        kb0 = stile([32, S], BF16, "kb0")
        qeo = stile([128, NB, 2, SQB], BF16, "qeo")   # [he|ho] x per-b [pair0|pair1]
        kdup = stile([128, S], BF16, "kdup")
        vsb = stile([128, NJ, 65], BF16, "vsb")       # [v | ones]
        yn = [stile([128, S], BF16, f"yn{m}") for m in range(2)]      # normalized y^T

        # ---- load everything (batched) ----
        nc.sync.dma_start(xt[:], xt_d[:])
        nc.sync.dma_start(wq[:], wq_d[:])
        nc.sync.dma_start(wkv[:], wkv_d[:])
        nc.sync.dma_start(wo[:], wo_d[:])
        nc.sync.dma_start(cb[:], cblk_d[:])
        nc.sync.dma_start(qlnb_s[:], qlnb_d[:])
        nc.vector.memset(vsb[:], 1.0)  # ones column at [:, j, 64]; 0:64 overwritten below
        nc.vector.memset(epsb[:], EPS)
        nc.vector.memset(zb[:], 0.0)

        # ======== phase 1: projections + rms factors + rope ========
        with (
            tc.tile_pool(name="pp", bufs=4, space=bass.MemorySpace.PSUM) as pp,
            tc.tile_pool(name="lns", bufs=2) as lns,
        ):
            # Q projection -> qsb (permuted: tileT = tops of 4 heads, tileB = bottoms)
            for m in range(2):
                pq = [pp.tile([128, 512], F32, name="pq", tag="pq", bufs=4) for _ in range(NS5)]
                for k in range(NK):
                    for n in range(NS5):
                        nc.tensor.matmul(
                            pq[n][:], wq[:, k * E + 128 * m:k * E + 128 * (m + 1)],
                            xt[:, k * S + 512 * n:k * S + 512 * (n + 1)],
                            start=(k == 0), stop=(k == NK - 1))
                for n in range(NS5):
                    sl = slice(512 * n, 512 * (n + 1))
                    nc.vector.tensor_copy(qsb[m][:, sl], pq[n][:])
                    nc.vector.tensor_mul(sqq[m][:, sl], qsb[m][:, sl], qsb[m][:, sl])
            # KV projection
            pkv = [pp.tile([128, 512], F32, name="pq", tag="pq", bufs=4) for _ in range(NS5)]
            for k in range(NK):
                for n in range(NS5):
                    nc.tensor.matmul(
                        pkv[n][:], wkv[:, k * 128:(k + 1) * 128],
                        xt[:, k * S + 512 * n:k * S + 512 * (n + 1)],
                        start=(k == 0), stop=(k == NK - 1))
            for n in range(NS5):
                sl = slice(512 * n, 512 * (n + 1))
                nc.vector.tensor_copy(kvsb[:, sl], pkv[n][:])
                nc.vector.tensor_mul(sqkv[:, sl], kvsb[0:64, sl], kvsb[0:64, sl])
                # v transpose: [64,128] slices -> [128,64]
                for t in range(4):
                    st_ = 4 * n + t
                    ptr = pp.tile([128, 64], BF16, name="ptr", tag="ptr", bufs=2)
                    nc.tensor.transpose(
                        ptr[:], kvsb[64:128, 128 * st_:128 * (st_ + 1)],
                        id128[64:128, 64:128])
                    nc.vector.tensor_copy(vsb[:, st_, 0:64], ptr[:])

            # rms factors: f = exp(-0.5*ln(ssq/HD + eps) + ln(gain/8))
            for n in range(NS5):
                sl = slice(512 * n, 512 * (n + 1))
                psq = pp.tile([4, 512], F32, name="psq", tag="psq", bufs=2)
                nc.tensor.matmul(psq[:], sel4, sqq[0][:, sl], start=True, stop=False)
                nc.tensor.matmul(psq[:], sel4, sqq[1][:, sl], start=False, stop=True)
                lnt = lns.tile([4, 512], F32, name="pln", tag="pln")
                nc.scalar.activation(lnt[:], psq[:], AF.Ln, scale=1.0 / HD, bias=epsb[0:4, :])
                nc.scalar.activation(fq[:, sl], lnt[:], AF.Exp, scale=-0.5,
                                     bias=qlnb_s[:, :])
                psk = pp.tile([1, 512], F32, name="psq", tag="psq", bufs=2)
                nc.tensor.matmul(psk[:], ones64col, sqkv[:, sl], start=True, stop=True)
                lnk = lns.tile([1, 512], F32, name="pln", tag="pln")
                nc.scalar.activation(lnk[:], psk[:], AF.Ln, scale=1.0 / HD, bias=epsb[0:1, :])
                nc.scalar.activation(fk[:, sl], lnk[:], AF.Exp, scale=-0.5, bias=zb[0:1, :])
                # broadcast factors along hd rows via PE
                pb = pp.tile([128, 512], F32, name="pq", tag="pq", bufs=4)
                nc.tensor.matmul(pb[:], bsel4, fq[:, sl], start=True, stop=True)
                nc.vector.tensor_copy(fbcq[:, sl], pb[:])
                pbk = pp.tile([64, 512], F32, name="pq", tag="pq", bufs=4)
                nc.tensor.matmul(pbk[:], ones64row, fk[:, sl], start=True, stop=True)
                nc.vector.tensor_copy(fbck[:, sl], pbk[:])

            # k bottom half shifted to partition base 0 (DVE ops need aligned bases)
            nc.sync.dma_start(kb0[:], kvsb[32:64, :])

            # rope + scale (DVE, bf16)
            with tc.tile_pool(name="rt", bufs=4) as rt:
                for n in range(NS5):
                    sl = slice(512 * n, 512 * (n + 1))
                    t1 = rt.tile([128, 512], BF16, name="t1", tag="t1")
                    t2 = rt.tile([128, 512], BF16, name="t2", tag="t2")
                    nc.vector.tensor_mul(t1[:], qsb[0][:, sl], cos4[:, sl])
                    nc.vector.tensor_mul(t2[:], qsb[1][:, sl], sin4[:, sl])
                    nc.vector.tensor_add(t1[:], t1[:], t2[:])
                    nc.vector.tensor_mul(qr[0][:, sl], t1[:], fbcq[:, sl])
                    u1 = rt.tile([128, 512], BF16, name="t1", tag="t1")
                    u2 = rt.tile([128, 512], BF16, name="t2", tag="t2")
                    nc.vector.tensor_mul(u1[:], qsb[0][:, sl], nsin4[:, sl])
                    nc.vector.tensor_mul(u2[:], qsb[1][:, sl], cos4[:, sl])
                    nc.vector.tensor_add(u1[:], u1[:], u2[:])
                    nc.vector.tensor_mul(qr[1][:, sl], u1[:], fbcq[:, sl])
                    k1 = rt.tile([32, 512], BF16, name="k1", tag="k1")
                    k2 = rt.tile([32, 512], BF16, name="k2", tag="k2")
                    nc.vector.tensor_mul(k1[:], kvsb[0:32, sl], cos4[0:32, sl])
                    nc.vector.tensor_mul(k2[:], kb0[:, sl], sin4[0:32, sl])
                    nc.vector.tensor_add(k1[:], k1[:], k2[:])
                    nc.vector.tensor_mul(kr[0][:, sl], k1[:], fbck[0:32, sl])
                    k3 = rt.tile([32, 512], BF16, name="k1", tag="k1")
                    k4 = rt.tile([32, 512], BF16, name="k2", tag="k2")
                    nc.vector.tensor_mul(k3[:], kvsb[0:32, sl], nsin4[0:32, sl])
                    nc.vector.tensor_mul(k4[:], kb0[:, sl], cos4[0:32, sl])
                    nc.vector.tensor_add(k3[:], k3[:], k4[:])
                    nc.vector.tensor_mul(kr[1][:, sl], k3[:], fbck[0:32, sl])

        # reassemble layouts (DMA partition/column moves)
        # qeo rows 0:64 = even-pair heads (h0 cols .,0 ; h2 cols .,1)
        # qeo rows 64:128 = odd-pair heads (h1 ; h3)
        for h, (rbase, pcol) in enumerate(((0, 0), (64, 0), (0, 1), (64, 1))):
            # h is the original head index; dst member column pcol, dst row base rbase
            src0 = qr[0][32 * h:32 * h + 32, :].rearrange("p (b s) -> p b s", b=NB)
            src1 = qr[1][32 * h:32 * h + 32, :].rearrange("p (b s) -> p b s", b=NB)
            nc.sync.dma_start(qeo[rbase:rbase + 32, :, pcol, :], src0)
            nc.sync.dma_start(qeo[rbase + 32:rbase + 64, :, pcol, :], src1)
        nc.sync.dma_start(kdup[0:32, :], kr[0][:])
        nc.sync.dma_start(kdup[32:64, :], kr[1][:])
        nc.sync.dma_start(kdup[64:96, :], kr[0][:])
        nc.sync.dma_start(kdup[96:128, :], kr[1][:])

        # ======== phase 2+3: attention with interleaved output projection ====
        with (
            tc.tile_pool(name="ps", bufs=2, space=bass.MemorySpace.PSUM) as ps,
            tc.tile_pool(name="py", bufs=2, space=bass.MemorySpace.PSUM) as py,
            tc.tile_pool(name="pa", bufs=3) as pa,
            tc.tile_pool(name="pn", bufs=2) as pn,
            tc.tile_pool(name="ob", bufs=2) as ob,
        ):
            for b in range(NB):
                sq = slice(SQB * b, SQB * (b + 1))
                jmax = 2 * b + 1
                yt = py.tile([65, 1024], F32, name="yt", tag="yt")
                for j in range(jmax + 1):
                    st = ps.tile([128, 1024], F32, name="st", tag="st")
                    jc = slice(128 * j, 128 * (j + 1))
                    nc.tensor.matmul(st[:, 0:512], kdup[0:64, jc],
                                     qeo[0:64, b, :, :],
                                     start=True, stop=True)
                    nc.tensor.matmul(st[:, 512:1024], kdup[64:128, jc],
                                     qeo[64:128, b, :, :],
                                     start=True, stop=True, skip_group_check=True)
                    pt = pa.tile([128, 1024], BF16, name="pt", tag="pt")
                    nc.scalar.activation(pt[:], st[:], AF.Exp, bias=zb[:, :])
                    if j >= 2 * b:
                        nc.vector.tensor_mul(pt[:], pt[:], mask_s[j - 2 * b])
                    nc.tensor.matmul(yt[:, 0:512], vsb[:, j, :], pt[:, 0:512],
                                     start=(j == 0), stop=(j == jmax))
                    nc.tensor.matmul(yt[:, 512:1024], vsb[:, j, :], pt[:, 512:1024],
                                     start=(j == 0), stop=(j == jmax),
                                     skip_group_check=True)

                # denominators: yt row 64 = sum exp per (head, query)
                dnb = pn.tile([1, 1024], F32, name="dnb", tag="dnb")
                nc.vector.tensor_copy(dnb[:], yt[64:65, :])
                rbb = pn.tile([64, 1024], F32, name="rbb", tag="rbb")
                nc.gpsimd.partition_broadcast(rbb[:], dnb[:])
                rbs = pn.tile([64, 1024], F32, name="rbs", tag="rbs")
                nc.vector.reciprocal_approx_fast(rbs[:], rbb[:])
                # normalize: yn0 rows = [h0 | h2], yn1 rows = [h1 | h3]
                for m in range(2):
                    nc.vector.tensor_mul(yn[m][0:64, sq],
                                         yt[0:64, 512 * m:512 * m + 256],
                                         rbs[:, 512 * m:512 * m + 256])
                    stg = pa.tile([64, 256], BF16, name="stg", tag="stg", bufs=2)
                    nc.vector.tensor_mul(stg[:],
                                         yt[0:64, 512 * m + 256:512 * m + 512],
                                         rbs[:, 512 * m + 256:512 * m + 512])
                    nc.sync.dma_start(yn[m][64:128, sq], stg[:])

                # output projection for the two 128-row s-tiles of this block
                for t in (2 * b, 2 * b + 1):
                    ssl = slice(128 * t, 128 * (t + 1))
                    pot = ps.tile([128, 1024], F32, name="st", tag="st")
                    for nh in range(2):
                        nsl = slice(512 * nh, 512 * (nh + 1))
                        for kk in range(2):
                            nc.tensor.matmul(
                                pot[:, nsl], yn[kk][:, ssl],
                                wo[:, kk * D + 512 * nh:kk * D + 512 * (nh + 1)],
                                start=(kk == 0), stop=(kk == 1))
                    ot = ob.tile([128, D], F32, name="ot", tag="ot")
                    nc.vector.tensor_copy(ot[:], pot[:])
                    nc.sync.dma_start(out_d[ssl, :], ot[:])

    nc.finalize()
    return nc


_NC = None


def _get_nc():
    global _NC
    if _NC is None:
        _NC = _build()
    return _NC


def _perm():
    tops = [h * 64 + i for h in range(HG) for i in range(32)]
    bots = [h * 64 + 32 + i for h in range(HG) for i in range(32)]
    return tops + bots


def build_inmaps(x, Wq, Wk, Wv, Wo, q_gain):
    x = np.asarray(x, dtype=np.float32)
    Wq = np.asarray(Wq, dtype=np.float32)
    Wk = np.asarray(Wk, dtype=np.float32)
    Wv = np.asarray(Wv, dtype=np.float32)
    Wo = np.asarray(Wo, dtype=np.float32)
    q_gain = np.asarray(q_gain, dtype=np.float32)

    perm = _perm()
    in_maps = []
    for c in range(8):
        dp, tp = divmod(c, 4)
        # xt[p, k*S+s] = x[dp][s, 128k+p]
        xt_p = np.ascontiguousarray(
            x[dp].reshape(S, NK, 128).transpose(2, 1, 0).reshape(128, NK * S)
        ).astype(BF16NP)
        wq_sel = Wq[tp * E:(tp + 1) * E].T[:, perm]          # [D, 256] permuted
        wq_p = np.ascontiguousarray(
            wq_sel.reshape(NK, 128, E).transpose(1, 0, 2).reshape(128, NK * E)
        ).astype(BF16NP)
        wk_sel = Wk[tp * HD:(tp + 1) * HD].T                  # [D, 64]
        wv_sel = Wv[tp * HD:(tp + 1) * HD].T
        wkv_sel = np.concatenate([wk_sel, wv_sel], axis=1)    # [D, 128]
        wkv_p = np.ascontiguousarray(
            wkv_sel.reshape(NK, 128, 128).transpose(1, 0, 2).reshape(128, NK * 128)
        ).astype(BF16NP)
        # wo rows ordered [h0, h2, h1, h3] to match yn stacking
        horder = [0, 2, 1, 3]
        wo_cols = np.concatenate(
            [np.arange(tp * E + h * HD, tp * E + (h + 1) * HD) for h in horder])
        wo_sel = Wo[:, wo_cols].T                             # [256, D]
        wo_p = np.ascontiguousarray(
            wo_sel.reshape(2, 128, D).transpose(1, 0, 2).reshape(128, 2 * D)
        ).astype(BF16NP)
        g = q_gain[tp * HG:(tp + 1) * HG].astype(np.float64)
        qlnb = np.log(np.maximum(g, 1e-30) / 8.0).astype(np.float32).reshape(4, 1)
        in_maps.append({
            "xt": xt_p, "wq": wq_p, "wkv": wkv_p, "wo": wo_p, "qlnb": qlnb,
        })
    return in_maps


def kernel(x, Wq, Wk, Wv, Wo, q_gain):
    in_maps = build_inmaps(x, Wq, Wk, Wv, Wo, q_gain)
    nc = _get_nc()
    res = run_bass_kernel_spmd(nc, in_maps, core_ids=list(range(8)))
    out = np.zeros((B, S, D), dtype=np.float32)
    for c in range(8):
        out[c // 4] += res.results[c]["out"]
    return out


# revision 15
# speedup vs baseline: 301.9815x; 1.3514x over previous
"""Trainium2 Bass kernel for causal GQA self-attention (B=2,S=2048,D=1024,H=16,HKV=4,HD=64).

Sharding: 8 cores = DP(2 over batch) x TP(4 over GQA groups).
Each core computes, for one batch element and one GQA group (4 q heads + 1 kv head),
the partial output  y_group @ Wo[:, group_cols].T  (row-sharded Wo).
Host sums the 4 TP partials per batch element.

v2: batched input DMAs, N=512 attention matmuls (head pairs packed in columns),
interleaved per-block output projection, ScalarE reserved for softmax exp,
GpSimd denominator broadcast, deeper PSUM pipelining.
"""

import sys
from contextlib import ExitStack

sys.path.insert(0, "/opt/trn_rl_repo")

import numpy as np
import ml_dtypes

import concourse.bass as bass
import concourse.bacc as bacc
import concourse.tile as tile
import concourse.mybir as mybir
from concourse.bass_utils import run_bass_kernel_spmd

BF16 = mybir.dt.bfloat16
F32 = mybir.dt.float32
AF = mybir.ActivationFunctionType
BF16NP = ml_dtypes.bfloat16

D, H, HKV, HD, B, S = 1024, 16, 4, 64, 2, 2048
HG = 4              # q heads per core
KV_DIM = HKV * HD   # 256
E = HG * HD         # 256 local q-proj dim
ROPE_BASE = 10000.0
EPS = float(np.finfo(np.float32).eps)

NK = D // 128       # 8 contraction tiles for qkv projections
SQB = 256           # sq block size in attention
NB = S // SQB       # 8 blocks
NJ = S // 128       # 16 sk tiles
NS5 = S // 512      # 4 n-tiles of 512 in projections

# const block column offsets (bf16 [128, CW])
_CO_COS = 0
_CO_SIN = _CO_COS + S
_CO_NSIN = _CO_SIN + S
_CO_M0 = _CO_NSIN + S
_CO_M1 = _CO_M0 + HG * SQB
_CO_ID = _CO_M1 + HG * SQB
_CO_SEL = _CO_ID + 128          # sel4 [128,4]
_CO_BSEL = _CO_SEL + 4          # bsel4 [4,128]
_CO_O64C = _CO_BSEL + 128       # ones64col [64,1]
_CO_O64R = _CO_O64C + 1         # ones64 row [1,64]
CW = _CO_O64R + 64


def _consts():
    """Constant block baked into the NEFF (same for every core): [128, CW] bf16."""
    blk = np.zeros((128, CW), dtype=BF16NP)
    i = np.arange(32, dtype=np.float64)
    inv_freq = 1.0 / (ROPE_BASE ** (2.0 * i / HD))
    pos = np.arange(S, dtype=np.float64)
    fr = pos[:, None] * inv_freq[None, :]          # [S, 32]
    cosT = np.cos(fr).T.astype(np.float32)          # [32, S]
    sinT = np.sin(fr).T.astype(np.float32)
    blk[:, _CO_COS:_CO_COS + S] = np.tile(cosT, (4, 1)).astype(BF16NP)
    blk[:, _CO_SIN:_CO_SIN + S] = np.tile(sinT, (4, 1)).astype(BF16NP)
    blk[:, _CO_NSIN:_CO_NSIN + S] = (-np.tile(sinT, (4, 1))).astype(BF16NP)

    # causal masks for diagonal sk-tiles: pattern p in {0,1}
    # valid iff c >= 128*p + r   (r: sk row 0..127, c: sq col 0..255)
    r = np.arange(128)[:, None]
    c = np.arange(SQB)[None, :]
    for p, co in ((0, _CO_M0), (1, _CO_M1)):
        m = (c >= 128 * p + r).astype(BF16NP)       # [128, 256]
        blk[:, co:co + HG * SQB] = np.tile(m, (1, HG))

    blk[:, _CO_ID:_CO_ID + 128] = np.eye(128, dtype=BF16NP)
    sel4 = np.zeros((128, 4), dtype=BF16NP)         # sumsq selector: tops of head h
    for h in range(4):
        sel4[32 * h:32 * h + 32, h] = 1.0
    blk[:, _CO_SEL:_CO_SEL + 4] = sel4
    bsel4 = np.zeros((4, 128), dtype=BF16NP)        # broadcast f[h] -> rows 32h..32h+32
    for h in range(4):
        bsel4[h, 32 * h:32 * h + 32] = 1.0
    blk[0:4, _CO_BSEL:_CO_BSEL + 128] = bsel4
    blk[0:64, _CO_O64C] = 1.0                       # ones64col [64,1]
    blk[0:1, _CO_O64R:_CO_O64R + 64] = 1.0          # ones64 row [1,64]
    return blk


def _build():
    nc = bacc.Bacc("TRN2", debug=False)

    xt_d = nc.dram_tensor("xt", [128, NK * S], BF16, kind="ExternalInput")
    wq_d = nc.dram_tensor("wq", [128, NK * E], BF16, kind="ExternalInput")
    wkv_d = nc.dram_tensor("wkv", [128, NK * 128], BF16, kind="ExternalInput")
    wo_d = nc.dram_tensor("wo", [128, 2 * D], BF16, kind="ExternalInput")
    qlnb_d = nc.dram_tensor("qlnb", [4, 1], F32, kind="ExternalInput")
    out_d = nc.dram_tensor("out", [S, D], F32, kind="ExternalOutput")

    cblk_d = nc.inline_tensor(_consts(), "cblk")

    with tile.TileContext(nc) as tc, ExitStack() as ctx:
        sp = ctx.enter_context(tc.tile_pool(name="static", bufs=1))

        def stile(shape, dt, tag):
            return sp.tile(shape, dt, name=tag, tag=tag)

        # ---- static SBUF tensors ----
        xt = stile([128, NK * S], BF16, "xt")
        wq = stile([128, NK * E], BF16, "wq")
        wkv = stile([128, NK * 128], BF16, "wkv")
        wo = stile([128, 2 * D], BF16, "wo")
        cb = stile([128, CW], BF16, "cb")
        qlnb_s = stile([4, 1], F32, "qlnb")
        epsb = stile([128, 1], F32, "epsb")
        zb = stile([128, 1], F32, "zb")

        # const views
        cos4 = cb[:, _CO_COS:_CO_COS + S]
        sin4 = cb[:, _CO_SIN:_CO_SIN + S]
        nsin4 = cb[:, _CO_NSIN:_CO_NSIN + S]
        mask_s = [cb[:, _CO_M0:_CO_M0 + HG * SQB], cb[:, _CO_M1:_CO_M1 + HG * SQB]]
        id128 = cb[:, _CO_ID:_CO_ID + 128]
        sel4 = cb[:, _CO_SEL:_CO_SEL + 4]
        bsel4 = cb[0:4, _CO_BSEL:_CO_BSEL + 128]
        ones64col = cb[0:64, _CO_O64C:_CO_O64C + 1]
        ones64row = cb[0:1, _CO_O64R:_CO_O64R + 64]

        qsb = [stile([128, S], BF16, f"qsb{m}") for m in range(2)]   # T/B packed
        kvsb = stile([128, S], BF16, "kvsb")                          # k(0:64) | v(64:128)
        sqq = [stile([128, S], BF16, f"sqq{m}") for m in range(2)]
        sqkv = stile([64, S], BF16, "sqkv")
        fq = stile([4, S], BF16, "fq")
        fk = stile([1, S], BF16, "fk")
        fbcq = stile([128, S], BF16, "fbcq")
        fbck = stile([64, S], BF16, "fbck")
        qr = [stile([128, S], BF16, f"qr{m}") for m in range(2)]      # rotated T/B
        kr = [stile([32, S], BF16, f"kr{m}") for m in range(2)]
        kb0 = stile([32, S], BF16, "kb0")
        qeo = stile([128, NB, 2, SQB], BF16, "qeo")   # [he|ho] x per-b [pair0|pair1]
        kdup = stile([128, S], BF16, "kdup")
        vsb = stile([128, NJ, 65], BF16, "vsb")       # [v | ones]
        yn = [stile([128, S], BF16, f"yn{m}") for m in range(2)]      # normalized y^T

        # ---- load everything (batched; xt chunked so projections start early) ----
        nc.sync.dma_start(cb[:], cblk_d[:])
        nc.sync.dma_start(wq[:], wq_d[:])
        nc.sync.dma_start(wkv[:], wkv_d[:])
        nc.sync.dma_start(qlnb_s[:], qlnb_d[:])
        for kc in range(4):
            sl = slice(kc * 2 * S, (kc + 1) * 2 * S)
            nc.sync.dma_start(xt[:, sl], xt_d[:, sl])
        nc.sync.dma_start(wo[:], wo_d[:])
        nc.vector.memset(vsb[:], 1.0)  # ones column at [:, j, 64]; 0:64 overwritten below
        nc.vector.memset(epsb[:], EPS)
        nc.vector.memset(zb[:], 0.0)

        # ======== phase 1: projections + rms factors + rope ========
        with (
            tc.tile_pool(name="pp", bufs=4, space=bass.MemorySpace.PSUM) as pp,
            tc.tile_pool(name="lns", bufs=2) as lns,
        ):
            # Q projection -> qsb (permuted: tileT = tops of 4 heads, tileB = bottoms)
            for m in range(2):
                pq = [pp.tile([128, 512], F32, name="pq", tag="pq", bufs=4) for _ in range(NS5)]
                for k in range(NK):
                    for n in range(NS5):
                        nc.tensor.matmul(
                            pq[n][:], wq[:, k * E + 128 * m:k * E + 128 * (m + 1)],
                            xt[:, k * S + 512 * n:k * S + 512 * (n + 1)],
                            start=(k == 0), stop=(k == NK - 1))
                for n in range(NS5):
                    sl = slice(512 * n, 512 * (n + 1))
                    nc.vector.tensor_copy(qsb[m][:, sl], pq[n][:])
                    nc.vector.tensor_mul(sqq[m][:, sl], qsb[m][:, sl], qsb[m][:, sl])
            # KV projection
            pkv = [pp.tile([128, 512], F32, name="pq", tag="pq", bufs=4) for _ in range(NS5)]
            for k in range(NK):
                for n in range(NS5):
                    nc.tensor.matmul(
                        pkv[n][:], wkv[:, k * 128:(k + 1) * 128],
                        xt[:, k * S + 512 * n:k * S + 512 * (n + 1)],
                        start=(k == 0), stop=(k == NK - 1))
            for n in range(NS5):
                sl = slice(512 * n, 512 * (n + 1))
                nc.vector.tensor_copy(kvsb[:, sl], pkv[n][:])
                nc.vector.tensor_mul(sqkv[:, sl], kvsb[0:64, sl], kvsb[0:64, sl])
                # v transpose: [64,128] slices -> [128,64]
                for t in range(4):
                    st_ = 4 * n + t
                    ptr = pp.tile([128, 64], BF16, name="ptr", tag="ptr", bufs=2)
                    nc.tensor.transpose(
                        ptr[:], kvsb[64:128, 128 * st_:128 * (st_ + 1)],
                        id128[64:128, 64:128])
                    nc.vector.tensor_copy(vsb[:, st_, 0:64], ptr[:])

            # rms factors: f = exp(-0.5*ln(ssq/HD + eps) + ln(gain/8))
            # (all Ln first, then all Exp: avoids ACT table thrash)
            lnq = stile([4, S], F32, "lnq")
            lnk = stile([1, S], F32, "lnk")
            for n in range(NS5):
                sl = slice(512 * n, 512 * (n + 1))
                psq = pp.tile([4, 512], F32, name="psq", tag="psq", bufs=2)
                nc.tensor.matmul(psq[:], sel4, sqq[0][:, sl], start=True, stop=False)
                nc.tensor.matmul(psq[:], sel4, sqq[1][:, sl], start=False, stop=True)
                nc.scalar.activation(lnq[:, sl], psq[:], AF.Ln, scale=1.0 / HD,
                                     bias=epsb[0:4, :])
                psk = pp.tile([1, 512], F32, name="psq", tag="psq", bufs=2)
                nc.tensor.matmul(psk[:], ones64col, sqkv[:, sl], start=True, stop=True)
                nc.scalar.activation(lnk[:, sl], psk[:], AF.Ln, scale=1.0 / HD,
                                     bias=epsb[0:1, :])
            for n in range(NS5):
                sl = slice(512 * n, 512 * (n + 1))
                nc.scalar.activation(fq[:, sl], lnq[:, sl], AF.Exp, scale=-0.5,
                                     bias=qlnb_s[:, :])
                nc.scalar.activation(fk[:, sl], lnk[:, sl], AF.Exp, scale=-0.5,
                                     bias=zb[0:1, :])
                # broadcast factors along hd rows via PE
                pb = pp.tile([128, 512], F32, name="pq", tag="pq", bufs=4)
                nc.tensor.matmul(pb[:], bsel4, fq[:, sl], start=True, stop=True)
                nc.vector.tensor_copy(fbcq[:, sl], pb[:])
                pbk = pp.tile([64, 512], F32, name="pq", tag="pq", bufs=4)
                nc.tensor.matmul(pbk[:], ones64row, fk[:, sl], start=True, stop=True)
                nc.vector.tensor_copy(fbck[:, sl], pbk[:])

            # rope + scale (DVE, bf16), with layout assembly DMAs interleaved
            # per S-half so attention can start as soon as the first half lands
            def assemble(half):
                hs = slice(half * (S // 2), (half + 1) * (S // 2))
                bh = slice(half * (NB // 2), (half + 1) * (NB // 2))
                for h, (rbase, pcol) in enumerate(((0, 0), (64, 0), (0, 1), (64, 1))):
                    src0 = qr[0][32 * h:32 * h + 32, hs].rearrange(
                        "p (b s) -> p b s", b=NB // 2)
                    src1 = qr[1][32 * h:32 * h + 32, hs].rearrange(
                        "p (b s) -> p b s", b=NB // 2)
                    nc.sync.dma_start(qeo[rbase:rbase + 32, bh, pcol, :], src0)
                    nc.sync.dma_start(qeo[rbase + 32:rbase + 64, bh, pcol, :], src1)
                nc.sync.dma_start(kdup[0:32, hs], kr[0][:, hs])
                nc.sync.dma_start(kdup[32:64, hs], kr[1][:, hs])
                nc.sync.dma_start(kdup[64:96, hs], kr[0][:, hs])
                nc.sync.dma_start(kdup[96:128, hs], kr[1][:, hs])

            # k bottom half shifted to partition base 0 (walrus requires
            # tensor_tensor operands to share the start partition)
            nc.sync.dma_start(kb0[:], kvsb[32:64, :])

            with tc.tile_pool(name="rt", bufs=4) as rt:
                for n in range(NS5):
                    sl = slice(512 * n, 512 * (n + 1))
                    t1 = rt.tile([128, 512], BF16, name="t1", tag="t1")
                    t2 = rt.tile([128, 512], BF16, name="t2", tag="t2")
                    nc.vector.tensor_mul(t1[:], qsb[0][:, sl], cos4[:, sl])
                    nc.vector.tensor_mul(t2[:], qsb[1][:, sl], sin4[:, sl])
                    nc.vector.tensor_add(t1[:], t1[:], t2[:])
                    nc.vector.tensor_mul(qr[0][:, sl], t1[:], fbcq[:, sl])
                    u1 = rt.tile([128, 512], BF16, name="t1", tag="t1")
                    u2 = rt.tile([128, 512], BF16, name="t2", tag="t2")
                    nc.vector.tensor_mul(u1[:], qsb[0][:, sl], nsin4[:, sl])
                    nc.vector.tensor_mul(u2[:], qsb[1][:, sl], cos4[:, sl])
                    nc.vector.tensor_add(u1[:], u1[:], u2[:])
                    nc.vector.tensor_mul(qr[1][:, sl], u1[:], fbcq[:, sl])
                    k1 = rt.tile([32, 512], BF16, name="k1", tag="k1")
                    k2 = rt.tile([32, 512], BF16, name="k2", tag="k2")
                    nc.vector.tensor_mul(k1[:], kvsb[0:32, sl], cos4[0:32, sl])
                    nc.vector.tensor_mul(k2[:], kb0[:, sl], sin4[0:32, sl])
                    nc.vector.tensor_add(k1[:], k1[:], k2[:])
                    nc.vector.tensor_mul(kr[0][:, sl], k1[:], fbck[0:32, sl])
                    k3 = rt.tile([32, 512], BF16, name="k1", tag="k1")
                    k4 = rt.tile([32, 512], BF16, name="k2", tag="k2")
                    nc.vector.tensor_mul(k3[:], kvsb[0:32, sl], nsin4[0:32, sl])
                    nc.vector.tensor_mul(k4[:], kb0[:, sl], cos4[0:32, sl])
                    nc.vector.tensor_add(k3[:], k3[:], k4[:])
                    nc.vector.tensor_mul(kr[1][:, sl], k3[:], fbck[0:32, sl])
                    if n == 1:
                        assemble(0)
                if True:
                    assemble(1)

        # ======== phase 2+3: attention with interleaved output projection ====
        with (
            tc.tile_pool(name="ps", bufs=2, space=bass.MemorySpace.PSUM) as ps,
            tc.tile_pool(name="py", bufs=1, space=bass.MemorySpace.PSUM) as py,
            tc.tile_pool(name="po", bufs=1, space=bass.MemorySpace.PSUM) as po,
            tc.tile_pool(name="pa", bufs=3) as pa,
            tc.tile_pool(name="pn", bufs=2) as pn,
            tc.tile_pool(name="ob", bufs=2) as ob,
        ):
            for b in range(NB):
                sq = slice(SQB * b, SQB * (b + 1))
                jmax = 2 * b + 1
                yt = py.tile([65, 1024], F32, name="yt", tag="yt")
                for j in range(jmax + 1):
                    st = ps.tile([128, 1024], F32, name="st", tag="st")
                    jc = slice(128 * j, 128 * (j + 1))
                    nc.tensor.matmul(st[:, 0:512], kdup[0:64, jc],
                                     qeo[0:64, b, :, :],
                                     start=True, stop=True)
                    nc.tensor.matmul(st[:, 512:1024], kdup[64:128, jc],
                                     qeo[64:128, b, :, :],
                                     start=True, stop=True, skip_group_check=True)
                    pt = pa.tile([128, 1024], BF16, name="pt", tag="pt")
                    nc.scalar.activation(pt[:], st[:], AF.Exp, bias=zb[:, :])
                    if j >= 2 * b:
                        nc.vector.tensor_mul(pt[:], pt[:], mask_s[j - 2 * b])
                    nc.tensor.matmul(yt[:, 0:512], vsb[:, j, :], pt[:, 0:512],
                                     start=(j == 0), stop=(j == jmax))
                    nc.tensor.matmul(yt[:, 512:1024], vsb[:, j, :], pt[:, 512:1024],
                                     start=(j == 0), stop=(j == jmax),
                                     skip_group_check=True)

                # evacuate yt to SBUF promptly so the single yt slot frees for b+1
                ytc = pn.tile([65, 1024], F32, name="ytc", tag="ytc")
                nc.vector.tensor_copy(ytc[:], yt[:])
                # denominators: ytc row 64 = sum exp per (head, query)
                # (staged to a base-0 tile: partition_broadcast reads partition 0)
                dnb = pn.tile([1, 1024], F32, name="dnb", tag="dnb")
                nc.vector.tensor_copy(dnb[:], ytc[64:65, :])
                rbb = pn.tile([64, 1024], F32, name="rbb", tag="rbb")
                nc.gpsimd.partition_broadcast(rbb[:], dnb[:])
                rbs = pn.tile([64, 1024], F32, name="rbs", tag="rbs")
                nc.vector.reciprocal_approx_fast(rbs[:], rbb[:])
                # normalize: yn0 rows = [h0 | h2], yn1 rows = [h1 | h3]
                # (upper halves staged at base 0 then DMA-moved: walrus requires
                # tensor_tensor dst/src start partitions to match)
                for m in range(2):
                    nc.vector.tensor_mul(yn[m][0:64, sq],
                                         ytc[0:64, 512 * m:512 * m + 256],
                                         rbs[:, 512 * m:512 * m + 256])
                    stg = pa.tile([64, 256], BF16, name="stg", tag="stg", bufs=2)
                    nc.vector.tensor_mul(stg[:],
                                         ytc[0:64, 512 * m + 256:512 * m + 512],
                                         rbs[:, 512 * m + 256:512 * m + 512])
                    nc.sync.dma_start(yn[m][64:128, sq], stg[:])

                # output projection for the two 128-row s-tiles of this block
                for t in (2 * b, 2 * b + 1):
                    ssl = slice(128 * t, 128 * (t + 1))
                    pot = po.tile([128, 1024], F32, name="pot", tag="pot")
                    for nh in range(2):
                        nsl = slice(512 * nh, 512 * (nh + 1))
                        for kk in range(2):
                            nc.tensor.matmul(
                                pot[:, nsl], yn[kk][:, ssl],
                                wo[:, kk * D + 512 * nh:kk * D + 512 * (nh + 1)],
                                start=(kk == 0), stop=(kk == 1))
                    ot = ob.tile([128, D], F32, name="ot", tag="ot")
                    nc.vector.tensor_copy(ot[:], pot[:])
                    nc.sync.dma_start(out_d[ssl, :], ot[:])

    nc.finalize()
    return nc


_NC = None


def _get_nc():
    global _NC
    if _NC is None:
        _NC = _build()
    return _NC


def _perm():
    tops = [h * 64 + i for h in range(HG) for i in range(32)]
    bots = [h * 64 + 32 + i for h in range(HG) for i in range(32)]
    return tops + bots


def build_inmaps(x, Wq, Wk, Wv, Wo, q_gain):
    x = np.asarray(x, dtype=np.float32)
    Wq = np.asarray(Wq, dtype=np.float32)
    Wk = np.asarray(Wk, dtype=np.float32)
    Wv = np.asarray(Wv, dtype=np.float32)
    Wo = np.asarray(Wo, dtype=np.float32)
    q_gain = np.asarray(q_gain, dtype=np.float32)

    perm = _perm()
    in_maps = []
    for c in range(8):
        dp, tp = divmod(c, 4)
        # xt[p, k*S+s] = x[dp][s, 128k+p]
        xt_p = np.ascontiguousarray(
            x[dp].reshape(S, NK, 128).transpose(2, 1, 0).reshape(128, NK * S)
        ).astype(BF16NP)
        wq_sel = Wq[tp * E:(tp + 1) * E].T[:, perm]          # [D, 256] permuted
        wq_p = np.ascontiguousarray(
            wq_sel.reshape(NK, 128, E).transpose(1, 0, 2).reshape(128, NK * E)
        ).astype(BF16NP)
        wk_sel = Wk[tp * HD:(tp + 1) * HD].T                  # [D, 64]
        wv_sel = Wv[tp * HD:(tp + 1) * HD].T
        wkv_sel = np.concatenate([wk_sel, wv_sel], axis=1)    # [D, 128]
        wkv_p = np.ascontiguousarray(
            wkv_sel.reshape(NK, 128, 128).transpose(1, 0, 2).reshape(128, NK * 128)
        ).astype(BF16NP)
        # wo rows ordered [h0, h2, h1, h3] to match yn stacking
        horder = [0, 2, 1, 3]
        wo_cols = np.concatenate(
            [np.arange(tp * E + h * HD, tp * E + (h + 1) * HD) for h in horder])
        wo_sel = Wo[:, wo_cols].T                             # [256, D]
        wo_p = np.ascontiguousarray(
            wo_sel.reshape(2, 128, D).transpose(1, 0, 2).reshape(128, 2 * D)
        ).astype(BF16NP)
        g = q_gain[tp * HG:(tp + 1) * HG].astype(np.float64)
        qlnb = np.log(np.maximum(g, 1e-30) / 8.0).astype(np.float32).reshape(4, 1)
        in_maps.append({
            "xt": xt_p, "wq": wq_p, "wkv": wkv_p, "wo": wo_p, "qlnb": qlnb,
        })
    return in_maps


def kernel(x, Wq, Wk, Wv, Wo, q_gain):
    in_maps = build_inmaps(x, Wq, Wk, Wv, Wo, q_gain)
    nc = _get_nc()
    res = run_bass_kernel_spmd(nc, in_maps, core_ids=list(range(8)))
    out = np.zeros((B, S, D), dtype=np.float32)
    for c in range(8):
        out[c // 4] += res.results[c]["out"]
    return out
